# revision 1
# baseline (speedup 1.0000x reference)
"""Causal MHA forward on 8 NeuronCores (Trainium2, Bass/Tile).

Sharding: batch (4) x head-half (2) -> 8 cores. Each core computes, for its
batch b and its 8 heads: QKV column-sliced projections, causal attention in
transposed-score layout (S^T[k, q] so softmax rowsums come from a
ones-augmented V matmul and no transposes are needed), and a partial dense
projection against the matching 512-row slice of dense_w. The host sums the
two partial dense outputs per batch and adds dense_b + wv_b @ dense_w
(valid because softmax rows sum to 1).
"""
import numpy as np

import concourse.bacc as bacc
import concourse.bass as bass
import concourse.tile as tile
import concourse.mybir as mybir
from concourse.bass_utils import run_bass_kernel_spmd

B, S, D, = 4, 2048, 1024
DC = 512           # per-core d slice (8 heads x 64)
H = 8              # heads per core
DH = 64
N_CORES = 8
F32 = mybir.dt.float32
AF = mybir.ActivationFunctionType
NEG = -1.0e9
SCALE = 1.0 / 32.0  # 1/sqrt(D_MODEL)
F32R = mybir.dt.float32r


def _r(ap):
    return ap


_CACHE = {}


def _build():
    nc = bacc.Bacc("TRN2", target_bir_lowering=False, debug=False,
                   num_devices=N_CORES)
    xt = nc.dram_tensor("xt", [D, S], F32R, kind="ExternalInput")
    wq = nc.dram_tensor("wq", [D, DC], F32R, kind="ExternalInput")
    wk = nc.dram_tensor("wk", [D, DC], F32R, kind="ExternalInput")
    wv = nc.dram_tensor("wv", [D, DC], F32R, kind="ExternalInput")
    qb = nc.dram_tensor("qb", [DC], F32, kind="ExternalInput")
    kb = nc.dram_tensor("kb", [DC], F32, kind="ExternalInput")
    wd = nc.dram_tensor("wd", [DC, D], F32R, kind="ExternalInput")
    band = nc.dram_tensor("band", [128, 2048], F32R, kind="ExternalInput")
    ident = nc.dram_tensor("ident", [128, 128], F32R, kind="ExternalInput")
    ones = nc.dram_tensor("ones", [128, 64], F32R, kind="ExternalInput")
    out = nc.dram_tensor("out", [S, D], F32, kind="ExternalOutput")

    with tile.TileContext(nc) as tc:
      with nc.allow_low_precision(reason="float32r is 4-byte storage; psum accum stays fp32"):
        with (
            tc.tile_pool(name="consts", bufs=1) as consts,
            tc.tile_pool(name="wqp", bufs=1) as wqp,
            tc.tile_pool(name="ktp", bufs=1) as ktp,
            tc.tile_pool(name="vap", bufs=1) as vap,
            tc.tile_pool(name="otp", bufs=1) as otp,
            tc.tile_pool(name="xts", bufs=3) as xtsp,
            tc.tile_pool(name="qtp", bufs=2) as qtp,
            tc.tile_pool(name="ptp", bufs=2) as ptp,
            tc.tile_pool(name="nrm", bufs=1) as nrm,
            tc.tile_pool(name="psb", bufs=2, space="PSUM") as psb,
            tc.tile_pool(name="psv", bufs=1, space="PSUM") as psv,
            tc.tile_pool(name="psm", bufs=1, space="PSUM") as psm,
        ):
            band_sb = consts.tile([128, 2048], F32R)
            nc.scalar.dma_start(out=band_sb, in_=band[:, :])
            id_sb = consts.tile([128, 128], F32R)
            nc.scalar.dma_start(out=id_sb, in_=ident[:, :])
            on_sb = consts.tile([128, 64], F32R)
            nc.scalar.dma_start(out=on_sb, in_=ones[:, :])
            qb_sb = consts.tile([128, 4], F32)
            nc.scalar.dma_start(out=qb_sb, in_=qb.ap().rearrange("(c p) -> p c", p=128))
            kb_sb = consts.tile([128, 4], F32)
            nc.scalar.dma_start(out=kb_sb, in_=kb.ap().rearrange("(c p) -> p c", p=128))

            wq_sb = wqp.tile([128, 8, DC], F32R)
            nc.scalar.dma_start(out=wq_sb, in_=wq.ap().rearrange("(c p) d -> p c d", p=128))

            kt = ktp.tile([128, 4, S], F32R)       # K^T, pair p rows = d 128p..
            va = vap.tile([128, 16, H, 65], F32R)  # V + ones col, per s-block
            ot = otp.tile([128, 4, S], F32R)       # O^T accumulated
            nc.vector.memset(va[:, :, :, 64:65].bitcast(F32), 1.0)

            # ---- Phase 1: K^T and V projections (stream x^T by s-chunk) ----
            with tc.tile_pool(name="wkv", bufs=1) as wkvp:
                wk_sb = wkvp.tile([128, 8, DC], F32R)
                wv_sb = wkvp.tile([128, 8, DC], F32R)

                def load_wk():
                    nc.sync.dma_start(out=wk_sb, in_=wk.ap().rearrange("(c p) d -> p c d", p=128))
                qts = {}

                def qproj(cc, xg_):
                    qt_ = qtp.tile([128, 4, 512], F32R, tag="qt", name=f"qt{cc}")
                    for p in range(4):
                        ps = psm.tile([128, 512], F32, tag="mm", name="qproj")
                        for i in range(8):
                            nc.tensor.matmul(ps, _r(wq_sb[:, i, 128 * p:128 * (p + 1)]),
                                             _r(xg_[i // 4][:, i % 4, :]),
                                             start=(i == 0), stop=(i == 7))
                        nc.vector.tensor_scalar_add(out=qt_[:, p, :], in0=ps,
                                                    scalar1=qb_sb[:, p:p + 1])
                    qts[cc] = qt_

                def load_xts(cc, order=None):
                    a = xtsp.tile([128, 4, 512], F32R, tag="xts", name="xts0")
                    b = xtsp.tile([128, 4, 512], F32R, tag="xts", name="xts1")
                    xv = xt.ap().rearrange("(i p) s -> p i s", p=128)
                    da = lambda: nc.sync.dma_start(
                        out=a, in_=xv[:, 0:4, 512 * cc:512 * (cc + 1)])
                    db = lambda: nc.sync.dma_start(
                        out=b, in_=xv[:, 4:8, 512 * cc:512 * (cc + 1)])
                    if order is None:
                        da(); db()
                    else:
                        da(); order(); db()
                    return [a, b]

                for sc in range(4):
                    if sc == 0:
                        xg = load_xts(sc, order=load_wk)
                        nc.sync.dma_start(out=wv_sb, in_=wv.ap().rearrange("(c p) d -> p c d", p=128))
                    else:
                        xg = load_xts(sc)
                    for p in range(4):
                        ps = psv.tile([128, 512], F32, tag="pvA", bufs=2, name="kvps")
                        for i in range(8):
                            nc.tensor.matmul(ps, _r(wk_sb[:, i, 128 * p:128 * (p + 1)]),
                                             _r(xg[i // 4][:, i % 4, :]),
                                             start=(i == 0), stop=(i == 7))
                        nc.vector.tensor_scalar_add(
                            out=kt[:, p, 512 * sc:512 * (sc + 1)], in0=ps,
                            scalar1=kb_sb[:, p:p + 1])
                    for sb_ in range(4):
                        ps = psv.tile([128, 512], F32, tag="pvA", bufs=2, name="kvps")
                        for i in range(8):
                            nc.tensor.matmul(ps, _r(xg[i // 4][:, i % 4, 128 * sb_:128 * (sb_ + 1)]),
                                             _r(wv_sb[:, i, :]), start=(i == 0), stop=(i == 7))
                        sblk = 4 * sc + sb_
                        nc.vector.tensor_copy(
                            out=va[:, sblk, :, 0:64],
                            in_=ps.rearrange("p (h d) -> p h d", h=H))
                    if sc == 0:
                        qproj(0, xg)

            # ---- Phase 2+3: attention + dense, chunk at a time ----
            with (
                tc.tile_pool(name="wdp", bufs=1) as wdp,
                tc.tile_pool(name="outp", bufs=3) as outp,
            ):
                wd_sb = wdp.tile([128, 4, D], F32R)
                nc.scalar.dma_start(out=wd_sb, in_=wd.ap().rearrange("(c p) d -> p c d", p=128))
                for c in range(4):
                    if c < 3:
                        qproj(c + 1, load_xts(c + 1))
                    qt = qts[c]
                    nj = 4 * c + 4
                    for p in range(4):
                        pvA = psv.tile([65, 512], F32, tag="pvA", bufs=2, name="pvA")
                        pvB = psv.tile([65, 512], F32, tag="pvB", bufs=1, name="pvB")
                        for j in range(nj):
                            sc_ps = psb.tile([128, 1024], F32)
                            is_band = j >= 4 * c
                            nc.tensor.matmul(sc_ps[:, 0:512],
                                             _r(kt[0:64, p, 128 * j:128 * (j + 1)]),
                                             _r(qt[0:64, p, :]),
                                             start=True, stop=not is_band)
                            nc.tensor.matmul(sc_ps[:, 512:1024],
                                             _r(kt[64:128, p, 128 * j:128 * (j + 1)]),
                                             _r(qt[64:128, p, :]),
                                             start=True, stop=not is_band)
                            if is_band:
                                jj = j - 4 * c
                                m = band_sb[:, 512 * jj:512 * (jj + 1)]
                                nc.tensor.matmul(sc_ps[:, 0:512], _r(id_sb), _r(m),
                                                 start=False, stop=True)
                                nc.tensor.matmul(sc_ps[:, 512:1024], _r(id_sb), _r(m),
                                                 start=False, stop=True)
                            pt = ptp.tile([128, 1024], F32R)
                            nc.scalar.activation(out=pt, in_=sc_ps, func=AF.Exp,
                                                 scale=SCALE)
                            nc.tensor.matmul(pvA, _r(va[:, j, 2 * p, :]), _r(pt[:, 0:512]),
                                             start=(j == 0), stop=(j == nj - 1))
                            nc.tensor.matmul(pvB, _r(va[:, j, 2 * p + 1, :]), _r(pt[:, 512:1024]),
                                             start=(j == 0), stop=(j == nj - 1))
                        rr = nrm.tile([128, 1024], F32R, tag="rr")
                        nc.vector.reciprocal(out=rr[64:65, 0:512], in_=pvA[64:65, :])
                        nc.vector.reciprocal(out=rr[64:65, 512:1024], in_=pvB[64:65, :])
                        bcA = psm.tile([128, 512], F32, tag="mm", name="bcA")
                        nc.tensor.matmul(bcA[0:64, :], _r(on_sb[64:65, :]), _r(rr[64:65, 0:512]),
                                         start=True, stop=True, tile_position=(64, 0))
                        nc.vector.tensor_copy(out=rr[0:64, 0:512], in_=bcA[0:64, :])
                        nc.vector.tensor_mul(out=ot[0:64, p, 512 * c:512 * (c + 1)],
                                             in0=pvA[0:64, :], in1=rr[0:64, 0:512])
                        nc.vector.tensor_copy(out=rr[0:64, 512:1024], in_=pvB[0:64, :])
                        bcB = psm.tile([128, 512], F32, tag="mm", name="bcB")
                        nc.tensor.matmul(bcB[0:64, :], _r(on_sb[64:65, :]), _r(rr[64:65, 512:1024]),
                                         start=True, stop=True, tile_position=(64, 0))
                        nc.vector.tensor_mul(out=rr[0:64, 512:1024], in0=rr[0:64, 512:1024],
                                             in1=bcB[0:64, :])
                        sh = psm.tile([128, 512], F32, tag="mm", name="sh")
                        nc.tensor.matmul(sh[64:128, :], id_sb[0:64, 0:64].bitcast(F32), rr[0:64, 512:1024].bitcast(F32),
                                         start=True, stop=True, tile_position=(0, 64))
                        nc.vector.tensor_copy(out=ot[64:128, p, 512 * c:512 * (c + 1)],
                                              in_=sh[64:128, :])
                    # dense for this chunk's 4 s-blocks
                    for sb_ in range(4 * c, 4 * c + 4):
                        os = outp.tile([128, 1024], F32)
                        for n in range(2):
                            ps = psv.tile([128, 512], F32, tag="pvA", bufs=2, name="dps")
                            for p in range(4):
                                nc.tensor.matmul(ps, _r(ot[:, p, 128 * sb_:128 * (sb_ + 1)]),
                                                 _r(wd_sb[:, p, 512 * n:512 * (n + 1)]),
                                                 start=(p == 0), stop=(p == 3))
                            nc.vector.tensor_copy(out=os[:, 512 * n:512 * (n + 1)], in_=ps)
                        nc.sync.dma_start(out=out[128 * sb_:128 * (sb_ + 1), :], in_=os)
    nc.compile()
    return nc


def get_nc():
    if "nc" not in _CACHE:
        _CACHE["nc"] = _build()
    return _CACHE["nc"]


def kernel(x, mask, wq_w, wq_b, wk_w, wk_b, wv_w, wv_b, dense_w, dense_b,
           _trace=False):
    x = np.asarray(x, dtype=np.float32)
    wq_w = np.asarray(wq_w, np.float32); wq_b = np.asarray(wq_b, np.float32)
    wk_w = np.asarray(wk_w, np.float32); wk_b = np.asarray(wk_b, np.float32)
    wv_w = np.asarray(wv_w, np.float32); wv_b = np.asarray(wv_b, np.float32)
    dense_w = np.asarray(dense_w, np.float32)
    dense_b = np.asarray(dense_b, np.float32)

    # causal band masks M_jj[k, q'] = -1e9 where q' < 128*jj + k, cols jj*512..
    band = np.zeros((128, 2048), np.float32)
    k_idx = np.arange(128)[:, None]
    q_idx = np.arange(512)[None, :]
    for jj in range(4):
        band[:, 512 * jj:512 * (jj + 1)] = np.where(q_idx < 128 * jj + k_idx, NEG, 0.0)
    ident = np.eye(128, dtype=np.float32)
    ones = np.ones((128, 64), np.float32)

    in_maps = []
    for core in range(N_CORES):
        b, hh = divmod(core, 2)
        sl = slice(DC * hh, DC * (hh + 1))
        in_maps.append({
            "xt": np.ascontiguousarray(x[b].T),
            "wq": np.ascontiguousarray(wq_w[:, sl]),
            "wk": np.ascontiguousarray(wk_w[:, sl]),
            "wv": np.ascontiguousarray(wv_w[:, sl]),
            "qb": np.ascontiguousarray(wq_b[sl]),
            "kb": np.ascontiguousarray(wk_b[sl]),
            "wd": np.ascontiguousarray(dense_w[sl, :]),
            "band": band, "ident": ident, "ones": ones,
        })
    nc = get_nc()
    res = run_bass_kernel_spmd(nc, in_maps, core_ids=list(range(N_CORES)),
                               trace=_trace)
    const = dense_b + wv_b @ dense_w  # bias terms deferred to host
    outs = np.empty((B, S, D), np.float32)
    for b in range(B):
        outs[b] = res.results[2 * b]["out"] + res.results[2 * b + 1]["out"] + const
    if _trace:
        kernel.last_result = res
    return outs



# revision 3
# speedup vs baseline: 1.4940x; 1.4940x over previous
"""Causal MHA forward on 8 NeuronCores (Trainium2, Bass/Tile).

Sharding: batch (4) x head-half (2) -> 8 cores; each core computes 8 heads of
one batch and a partial dense projection (512 rows of dense_w); host sums the
two partials per batch and adds dense_b + wv_b @ dense_w.

Per-core compute layout:
- Q/K projections and the score matmuls run in fp8(e4m3) DoubleRow perf mode
  (2 contraction tiles per instruction, 0.5 PE cycles/row). The d=64 head dim
  is folded into DoubleRow's 2-slot axis by permuting wq/wk columns on the
  host so each head's two 32-wide d-halves land on the same 32 partitions at
  slot 0/1 of the Q^T/K^T sbuf tiles. Q/K are scaled x4 (weights host-scaled)
  so fp8 quantization stays in the normal range; exp scale becomes 1/512.
- The causal band mask is added into the score psum with a DoubleRow matmul
  (identity fp8e4 lhsT x band fp8e5 rhs, -7168 = -14*512 at masked slots).
- V projection, P (exp output), PV and dense run in bf16 (fp8 there fails the
  2e-2 tolerance: output max sits at early seq positions with concentrated
  attention, where V/O/Wd quantization error passes straight through).
- PV uses P^T as lhsT so O comes out in [q, d] layout (65 moving rows per
  128x128 tile incl. the appended ones column of V for rowsums); rowsums land
  per-partition so softmax normalization is a reciprocal + broadcast multiply
  fused with the psum->sbuf bf16 copy.
- O [q, d] -> O^T [d, q] via DMA xbar transpose (14ns/tile on DMA engines),
  then dense computes Y^T = Wd^T-tiles @ O^T; host transposes the output.
- All psum flows through one shared pool of four 2-bank tiles; PV packs
  8 (head, q-sub) 65-col accumulation regions into 2 banks using explicit
  start-group control (first write per bank uses start=True; the whole-bank
  pending-zero then covers the sibling regions).
"""
import numpy as np
import ml_dtypes

import concourse.bacc as bacc
import concourse.bass as bass
import concourse.tile as tile
import concourse.mybir as mybir
from concourse.bass_utils import run_bass_kernel_spmd

B, S, D = 4, 2048, 1024
DC = 512           # per-core d slice (8 heads x 64)
H = 8              # heads per core
DH = 64
N_CORES = 8
F32 = mybir.dt.float32
BF16 = mybir.dt.bfloat16
F8E4 = mybir.dt.float8e4
F8E5 = mybir.dt.float8e5
AF = mybir.ActivationFunctionType
DR = mybir.MatmulPerfMode.DoubleRow
QKSCALE = 4.0              # host scales wq/wk (and biases) by this
SCALE = 1.0 / (32.0 * QKSCALE * QKSCALE)   # exp scale (base 1/sqrt(1024))
MASKVAL = -14.0 / SCALE    # -7168, exactly representable in e5m2

E4NP = ml_dtypes.float8_e4m3
E5NP = ml_dtypes.float8_e5m2
BFNP = ml_dtypes.bfloat16

_CACHE = {}


def _build():
    nc = bacc.Bacc("TRN2", target_bir_lowering=False, debug=False,
                   num_devices=N_CORES)
    xt8 = nc.dram_tensor("xt8", [D, S], F8E4, kind="ExternalInput")
    xtb = nc.dram_tensor("xtb", [D, S], BF16, kind="ExternalInput")
    wq8 = nc.dram_tensor("wq8", [D, DC], F8E4, kind="ExternalInput")
    wk8 = nc.dram_tensor("wk8", [D, DC], F8E4, kind="ExternalInput")
    wvb = nc.dram_tensor("wvb", [D, DC], BF16, kind="ExternalInput")
    wdb = nc.dram_tensor("wdb", [DC, D], BF16, kind="ExternalInput")
    qb4 = nc.dram_tensor("qb4", [DC], F32, kind="ExternalInput")
    kb4 = nc.dram_tensor("kb4", [DC], F32, kind="ExternalInput")
    band = nc.dram_tensor("band", [128, 2, 2048], F8E5, kind="ExternalInput")
    idz = nc.dram_tensor("idz", [128, 2, 128], F8E4, kind="ExternalInput")
    yt = nc.dram_tensor("yt", [D, S], F32, kind="ExternalOutput")

    with tile.TileContext(nc) as tc:
      with nc.allow_low_precision(reason="bf16/fp8 inputs; all matmul accum in fp32 psum"):
        with (
            tc.tile_pool(name="consts", bufs=1) as consts,
            tc.tile_pool(name="wp", bufs=1) as wp,
            tc.tile_pool(name="qkp", bufs=1) as qkp,
            tc.tile_pool(name="vap", bufs=1) as vap,
            tc.tile_pool(name="otp", bufs=1) as otp,
            tc.tile_pool(name="xs", bufs=2) as xsp,
            tc.tile_pool(name="ptp", bufs=2) as ptp,
            tc.tile_pool(name="osb", bufs=2) as osbp,
            tc.tile_pool(name="rsp", bufs=2) as rsp,
            tc.tile_pool(name="dst", bufs=3) as dstp,
            tc.tile_pool(name="ps", bufs=4, space="PSUM") as psp,
        ):
            band_sb = consts.tile([128, 2, 2048], F8E5)
            nc.scalar.dma_start(out=band_sb, in_=band.ap())
            idz_sb = consts.tile([128, 2, 128], F8E4)
            nc.scalar.dma_start(out=idz_sb, in_=idz.ap())
            qb_sb = consts.tile([128, 4], F32)
            nc.scalar.dma_start(out=qb_sb, in_=qb4.ap().rearrange("(c p) -> p c", p=128))
            kb_sb = consts.tile([128, 4], F32)
            nc.scalar.dma_start(out=kb_sb, in_=kb4.ap().rearrange("(c p) -> p c", p=128))

            wq_sb = wp.tile([128, 8, DC], F8E4)
            nc.scalar.dma_start(out=wq_sb, in_=wq8.ap().rearrange("(c p) d -> p c d", p=128))
            wk_sb = wp.tile([128, 8, DC], F8E4)
            nc.scalar.dma_start(out=wk_sb, in_=wk8.ap().rearrange("(c p) d -> p c d", p=128))
            wv_sb = wp.tile([128, 8, DC], BF16)
            nc.scalar.dma_start(out=wv_sb, in_=wvb.ap().rearrange("(c p) d -> p c d", p=128))
            wd_sb = wp.tile([128, 4, D], BF16)
            nc.scalar.dma_start(out=wd_sb, in_=wdb.ap().rearrange("(c p) o -> p c o", p=128))

            # Q^T/K^T in fp8, partition = 32*(head%4) + d%32, dims (headgroup, d-half, s)
            q_sb = qkp.tile([128, 2, 2, S], F8E4)
            k_sb = qkp.tile([128, 2, 2, S], F8E4)
            # V + ones column per 128-row s-block
            va = vap.tile([128, 16, H, DH + 1], BF16)
            nc.vector.memset(va[:, :, :, DH:DH + 1], 1.0)
            ot = otp.tile([128, 4, S], BF16)   # O^T, row d = 128*dim1 + partition

            def load_x(c):
                a8 = xsp.tile([128, 8, 512], F8E4, tag="x8", name=f"x8_{c}")
                ab = xsp.tile([128, 8, 512], BF16, tag="xb", name=f"xb_{c}")
                nc.sync.dma_start(
                    out=a8, in_=xt8.ap().rearrange("(c p) s -> p c s", p=128)[:, :, 512 * c:512 * (c + 1)])
                nc.sync.dma_start(
                    out=ab, in_=xtb.ap().rearrange("(c p) s -> p c s", p=128)[:, :, 512 * c:512 * (c + 1)])
                return a8, ab

            def proj(c, x8, xb):
                sl = slice(512 * c, 512 * (c + 1))
                for p in range(4):
                    ps = psp.tile([128, 1024], F32, tag="ps", name=f"qk{c}{p}")
                    for i in range(4):
                        nc.tensor.matmul(ps[:, 0:512],
                                         wq_sb[:, 2 * i:2 * i + 2, 128 * p:128 * (p + 1)],
                                         x8[:, 2 * i:2 * i + 2, :],
                                         start=(i == 0), stop=(i == 3), perf_mode=DR)
                    for i in range(4):
                        nc.tensor.matmul(ps[:, 512:1024],
                                         wk_sb[:, 2 * i:2 * i + 2, 128 * p:128 * (p + 1)],
                                         x8[:, 2 * i:2 * i + 2, :],
                                         start=(i == 0), stop=(i == 3), perf_mode=DR)
                    nc.vector.tensor_scalar_add(out=q_sb[:, p // 2, p % 2, sl],
                                                in0=ps[:, 0:512], scalar1=qb_sb[:, p:p + 1])
                    nc.vector.tensor_scalar_add(out=k_sb[:, p // 2, p % 2, sl],
                                                in0=ps[:, 512:1024], scalar1=kb_sb[:, p:p + 1])
                for half in range(2):
                    ps = psp.tile([128, 1024], F32, tag="ps", name=f"v{c}{half}")
                    for g2 in range(2):
                        sb_ = 2 * half + g2
                        for i in range(8):
                            nc.tensor.matmul(ps[:, 512 * g2:512 * (g2 + 1)],
                                             xb[:, i, 128 * sb_:128 * (sb_ + 1)],
                                             wv_sb[:, i, :],
                                             start=(i == 0), stop=(i == 7))
                        nc.vector.tensor_copy(
                            out=va[:, 4 * c + sb_, :, 0:DH],
                            in_=ps[:, 512 * g2:512 * (g2 + 1)].rearrange("p (h d) -> p h d", h=H))

            def scores_p(c, p, ptc):
                nj = 4 * (c + 1)
                for j in range(nj):
                    sc = psp.tile([128, 1024], F32, tag="ps", name=f"sc{c}{p}{j}")
                    isb = j >= 4 * c
                    for hi in range(2):
                        h = 2 * p + hi
                        g, hh = h // 4, h % 4
                        lo = 32 * hh
                        nc.tensor.matmul(sc[:, 512 * hi:512 * (hi + 1)],
                                         k_sb[lo:lo + 32, g, :, 128 * j:128 * (j + 1)],
                                         q_sb[lo:lo + 32, g, :, 512 * c:512 * (c + 1)],
                                         start=True, stop=not isb, perf_mode=DR,
                                         tile_position=(lo, 0))
                        if isb:
                            jj = j - 4 * c
                            nc.tensor.matmul(sc[:, 512 * hi:512 * (hi + 1)],
                                             idz_sb[:, :, :],
                                             band_sb[:, :, 512 * jj:512 * (jj + 1)],
                                             start=False, stop=True, perf_mode=DR)
                    nc.scalar.activation(out=ptc[:, j, :], in_=sc, func=AF.Exp,
                                         scale=SCALE)

            def pv_p(c, p, ptc, pvt):
                nj = 4 * (c + 1)
                for j in range(nj):
                    for hi in range(2):
                        h = 2 * p + hi
                        for qs in range(4):
                            b, r = qs // 2, qs % 2
                            off = 130 * r + 65 * hi
                            nc.tensor.matmul(
                                pvt[:, b, off:off + 65],
                                ptc[:, j, 512 * hi + 128 * qs:512 * hi + 128 * qs + 128],
                                va[:, j, h, :],
                                start=(j == 0 and hi == 0 and r == 0),
                                stop=(j == nj - 1),
                                skip_group_check=True)

            def norm_p(c, p, pvt, osb):
                rs = rsp.tile([128, 2, 2, 2, 1], F32, tag="rs", name=f"rs{c}{p}")
                ov = osb.rearrange("p q (h d) -> p q h d", d=DH)
                for b in range(2):
                    pv_v = pvt[:, b, 0:260].rearrange("p (r h e) -> p r h e", r=2, h=2)
                    rsv = rs[:, b, :, :, :]
                    nc.vector.reciprocal(out=rsv, in_=pv_v[:, :, :, DH:DH + 1])
                    nc.vector.tensor_mul(
                        out=ov[:, 2 * b:2 * b + 2, 2 * p:2 * p + 2, :],
                        in0=pv_v[:, :, :, 0:DH],
                        in1=rsv.broadcast_to([128, 2, 2, DH]))

            def transposes(c, osb):
                for qs in range(4):
                    nc.sync.dma_start_transpose(
                        out=ot[:, :, 512 * c + 128 * qs:512 * c + 128 * (qs + 1)],
                        in_=osb[:, qs, :].rearrange("p a -> p a"))

            def dense(c):
                for ob2 in range(4):
                    ps = psp.tile([128, 1024], F32, tag="ps", name=f"d{c}{ob2}")
                    st = dstp.tile([128, 1024], F32, tag="dst")
                    for g2 in range(2):
                        ob = 2 * ob2 + g2
                        for c4 in range(4):
                            nc.tensor.matmul(ps[:, 512 * g2:512 * (g2 + 1)],
                                             wd_sb[:, c4, 128 * ob:128 * (ob + 1)],
                                             ot[:, c4, 512 * c:512 * (c + 1)],
                                             start=(c4 == 0), stop=(c4 == 3))
                        nc.vector.tensor_copy(out=st[:, 512 * g2:512 * (g2 + 1)],
                                              in_=ps[:, 512 * g2:512 * (g2 + 1)])
                    nc.sync.dma_start(
                        out=yt.ap().rearrange("(c p) s -> p c s", p=128)[:, 2 * ob2:2 * ob2 + 2, 512 * c:512 * (c + 1)],
                        in_=st.rearrange("p (a b) -> p a b", a=2))

            x8, xb = load_x(0)
            proj(0, x8, xb)
            for c in range(4):
                ptc = ptp.tile([128, 16, 1024], BF16, tag="pt", name=f"pt{c}")
                if c < 3:
                    xn8, xnb = load_x(c + 1)
                for p in range(4):
                    scores_p(c, p, ptc)
                    if p == 0 and c < 3:
                        proj(c + 1, xn8, xnb)
                    if p == 1 and c > 0:
                        dense(c - 1)
                    pvt = psp.tile([128, 2, 512], F32, tag="ps", name=f"pv{c}{p}")
                    pv_p(c, p, ptc, pvt)
                    osb = (osbp.tile([128, 4, 512], BF16, tag="osb", name=f"osb{c}")
                           if p == 0 else osb)
                    norm_p(c, p, pvt, osb)
                transposes(c, osb)
            dense(3)
    nc.compile()
    return nc


def get_nc():
    if "nc" not in _CACHE:
        _CACHE["nc"] = _build()
    return _CACHE["nc"]


def _perm():
    # new column order: (headgroup g, d-half, head-in-group hh, d%32)
    p = np.empty(DC, np.int64)
    pos = 0
    for g in range(2):
        for half in range(2):
            for hh in range(4):
                for d in range(32):
                    p[pos] = (g * 4 + hh) * DH + half * 32 + d
                    pos += 1
    return p


def kernel(x, mask, wq_w, wq_b, wk_w, wk_b, wv_w, wv_b, dense_w, dense_b,
           _trace=False):
    x = np.asarray(x, dtype=np.float32)
    wq_w = np.asarray(wq_w, np.float32); wq_b = np.asarray(wq_b, np.float32)
    wk_w = np.asarray(wk_w, np.float32); wk_b = np.asarray(wk_b, np.float32)
    wv_w = np.asarray(wv_w, np.float32); wv_b = np.asarray(wv_b, np.float32)
    dense_w = np.asarray(dense_w, np.float32)
    dense_b = np.asarray(dense_b, np.float32)

    perm = _perm()
    # causal band: band[k, 0, 512*jj + q'] = MASKVAL where q' < 128*jj + k
    band = np.zeros((128, 2, 2048), np.float32)
    k_idx = np.arange(128)[:, None]
    q_idx = np.arange(512)[None, :]
    for jj in range(4):
        band[:, 0, 512 * jj:512 * (jj + 1)] = np.where(
            q_idx < 128 * jj + k_idx, MASKVAL, 0.0)
    band = band.astype(E5NP)
    idz = np.zeros((128, 2, 128), np.float32)
    idz[:, 0, :] = np.eye(128, dtype=np.float32)
    idz = idz.astype(E4NP)

    in_maps = []
    for core in range(N_CORES):
        b, hh = divmod(core, 2)
        sl = slice(DC * hh, DC * (hh + 1))
        xt = np.ascontiguousarray(x[b].T)
        in_maps.append({
            "xt8": xt.astype(E4NP),
            "xtb": xt.astype(BFNP),
            "wq8": np.ascontiguousarray((QKSCALE * wq_w[:, sl])[:, perm]).astype(E4NP),
            "wk8": np.ascontiguousarray((QKSCALE * wk_w[:, sl])[:, perm]).astype(E4NP),
            "wvb": np.ascontiguousarray(wv_w[:, sl]).astype(BFNP),
            "wdb": np.ascontiguousarray(dense_w[sl, :]).astype(BFNP),
            "qb4": np.ascontiguousarray((QKSCALE * wq_b[sl])[perm]),
            "kb4": np.ascontiguousarray((QKSCALE * wk_b[sl])[perm]),
            "band": band, "idz": idz,
        })
    nc = get_nc()
    res = run_bass_kernel_spmd(nc, in_maps, core_ids=list(range(N_CORES)),
                               trace=_trace)
    const = dense_b + wv_b @ dense_w  # bias terms deferred to host
    outs = np.empty((B, S, D), np.float32)
    for b in range(B):
        outs[b] = (res.results[2 * b]["yt"] + res.results[2 * b + 1]["yt"]).T + const
    if _trace:
        kernel.last_result = res
    return outs


# revision 50
# speedup vs baseline: 2.0777x; 1.3906x over previous
"""Causal MHA forward on 8 NeuronCores (Trainium2, Bass/Tile).

Sharding: batch (4) x head-half (2) -> 8 cores; each core computes 8 heads of
one batch and a partial dense projection (512 rows of dense_w); host sums the
two partials per batch and adds dense_b + wv_b @ dense_w.

Per-core compute layout:
- Q/K projections and the score matmuls run in fp8(e4m3) DoubleRow perf mode
  (2 contraction tiles per instruction, 0.5 PE cycles/row). The d=64 head dim
  is folded into DoubleRow's 2-slot axis by permuting wq/wk columns on the
  host so each head's two 32-wide d-halves land on the same 32 partitions at
  slot 0/1 of the Q^T/K^T sbuf tiles. Q/K are scaled x4 (weights host-scaled)
  so fp8 quantization stays in the normal range; exp scale becomes 1/512.
- The causal band mask is added into the score psum with a DoubleRow matmul
  (identity fp8e4 lhsT x band fp8e5 rhs, -7168 = -14*512 at masked slots).
- V projection, P (exp output), PV and dense run in bf16 (fp8 there fails the
  2e-2 tolerance: output max sits at early seq positions with concentrated
  attention, where V/O/Wd quantization error passes straight through).
- PV uses P^T as lhsT so O comes out in [q, d] layout (65 moving rows per
  128x128 tile incl. the appended ones column of V for rowsums); rowsums land
  per-partition so softmax normalization is a reciprocal + broadcast multiply
  fused with the psum->sbuf bf16 copy.
- O [q, d] -> O^T [d, q] via DMA xbar transpose (14ns/tile on DMA engines),
  then dense computes Y^T = Wd^T-tiles @ O^T; host transposes the output.
- All psum flows through one shared pool of four 2-bank tiles; PV packs
  8 (head, q-sub) 65-col accumulation regions into 2 banks using explicit
  start-group control (first write per bank uses start=True; the whole-bank
  pending-zero then covers the sibling regions).
"""
import numpy as np
import ml_dtypes

import concourse.bacc as bacc
import concourse.bass as bass
import concourse.tile as tile
import concourse.mybir as mybir
from concourse.bass_utils import run_bass_kernel_spmd

B, S, D = 4, 2048, 1024
DC = 512           # per-core d slice (8 heads x 64)
H = 8              # heads per core
DH = 64
N_CORES = 8
F32 = mybir.dt.float32
BF16 = mybir.dt.bfloat16
I16 = mybir.dt.int16
F8E4 = mybir.dt.float8e4
F8E5 = mybir.dt.float8e5
AF = mybir.ActivationFunctionType
ALU = mybir.AluOpType
DR = mybir.MatmulPerfMode.DoubleRow
QKSCALE = 4.0              # host scales wq/wk (and biases) by this
SCALE = 1.0 / (32.0 * QKSCALE * QKSCALE)   # exp scale (base 1/sqrt(1024))
MASKVAL = -14.0 / SCALE    # -7168, exactly representable in e5m2
# Schraudolph fast-exp (bf16 bits via int16 mult-add on DVE/GpSimd):
# bf16_bits(exp(s*SCALE)) ~= int16(s * SCH_A + SCH_B)
SCH_A = float(np.float32(128.0 / np.log(2.0)) * np.float32(SCALE))
SCH_B = 16250.0

E4NP = ml_dtypes.float8_e4m3
E5NP = ml_dtypes.float8_e5m2
BFNP = ml_dtypes.bfloat16

_CACHE = {}


def _build(nobias=False):
    nc = bacc.Bacc("TRN2", target_bir_lowering=False, debug=False,
                   num_devices=N_CORES)
    xt8 = nc.dram_tensor("xt8", [D, S], F8E4, kind="ExternalInput")
    xtb = nc.dram_tensor("xtb", [D, S], BF16, kind="ExternalInput")
    wq8 = nc.dram_tensor("wq8", [D, DC], F8E4, kind="ExternalInput")
    wk8 = nc.dram_tensor("wk8", [D, DC], F8E4, kind="ExternalInput")
    wvb = nc.dram_tensor("wvb", [D, DC], BF16, kind="ExternalInput")
    wdb = nc.dram_tensor("wdb", [DC, D], BF16, kind="ExternalInput")
    qkb4 = nc.dram_tensor("qkb4", [1, 2 * DC], mybir.dt.float32r, kind="ExternalInput")
    onesr = nc.dram_tensor("onesr", [1, 512], mybir.dt.float32r, kind="ExternalInput")
    band = nc.dram_tensor("band", [128, 2048], F8E5, kind="ExternalInput")
    idz = nc.dram_tensor("idz", [128, 2, 128], F8E4, kind="ExternalInput")
    bb = nc.dram_tensor("bb", [128, 4, 512], I16, kind="ExternalInput")
    yt = nc.dram_tensor("yt", [D, S], F32, kind="ExternalOutput")

    # Greedy Act/DVE assignment of exp tiles and psum->sbuf copies (GpSimd
    # cannot touch PSUM, so only these two engines can drain it). DVE fuses
    # the causal-mask add into its Schraudolph op; Act band tiles pay a small
    # PE mask-matmul cost. Band tiles are column-clipped: cols q' < 128*jj are
    # fully masked, so scores/exp skip them and PV skips whole masked q-subs.
    expeng = {}
    cpeng = {}
    for c_ in range(4):
        load = {"act": 0.0, "dve": 16 * 258.0 / 4 + 2 * 392.0 * 4}
        cost = {"act": 1038.0, "dve": 1245.0}

        def pick(extra_act=0.0, rows=1024):
            scale = rows / 1024.0
            a = load["act"] + (scale * (853.0 + 185.0 / max(scale, 0.2))) + extra_act
            d = load["dve"] + (scale * (1067.0 + 178.0 / max(scale, 0.2)))
            if a <= d:
                load["act"] += scale * (853.0 + 185.0 / max(scale, 0.2))
                return "act"
            load["dve"] += scale * (1067.0 + 178.0 / max(scale, 0.2))
            return "dve"

        def pick_copy(key):
            import os
            if os.environ.get("FORCE_COPY_ENG"):
                cpeng[key] = os.environ["FORCE_COPY_ENG"]
                return
            a = load["act"] + 611.0
            d = load["dve"] + 658.0
            if a <= d:
                load["act"] += 611.0
                cpeng[key] = "act"
            else:
                load["dve"] += 658.0
                cpeng[key] = "dve"

        for p_ in range(4):
            for g2_ in range(2):
                pick_copy(("qk", c_, p_, g2_))
        for h_ in range(2):
            pick_copy(("v", c_, h_, 0))
            pick_copy(("v", c_, h_, 1))
        for ob2_ in range(4):
            pick_copy(("d", c_, ob2_, 0))
            pick_copy(("d", c_, ob2_, 1))
        for p_ in range(4):
            for j_ in range(4 * (c_ + 1)):
                jj_ = j_ - 4 * c_
                rows = 1024 if jj_ < 0 else 2 * (512 - 128 * jj_)
                expeng[(c_, p_, j_)] = pick(extra_act=(150.0 if jj_ >= 0 else 0.0),
                                            rows=rows)
                import os
                if os.environ.get('FORCE_EXP_ENG'):
                    expeng[(c_, p_, j_)] = os.environ['FORCE_EXP_ENG']

    with tile.TileContext(nc) as tc:
      with nc.allow_low_precision(reason="bf16/fp8 inputs; all matmul accum in fp32 psum"):
        with (
            tc.tile_pool(name="consts", bufs=1) as consts,
            tc.tile_pool(name="wp", bufs=1) as wp,
            tc.tile_pool(name="qkp", bufs=1) as qkp,
            tc.tile_pool(name="vap", bufs=1) as vap,
            tc.tile_pool(name="otp", bufs=1) as otp,
            tc.tile_pool(name="xs", bufs=1) as xsp,
            tc.tile_pool(name="ptp", bufs=3) as ptp,
            tc.tile_pool(name="osb", bufs=3) as osbp,
            tc.tile_pool(name="rsp", bufs=4) as rsp,
            tc.tile_pool(name="dst", bufs=4) as dstp,
            tc.tile_pool(name="ps", bufs=3, space="PSUM") as psp,
            tc.tile_pool(name="pv", bufs=1, space="PSUM") as pvp,
        ):
            x8_0 = xsp.tile([128, 8, 512], F8E4, tag="x8", name="x8_0")
            nc.sync.dma_start(out=x8_0, in_=xt8.ap().rearrange("(c p) s -> p c s", p=128)[:, :, 0:512])
            # DMA issue order follows first-use: x+wq/wk (proj0), wv/xb,
            # then mask consts (chunk-0 scores), biases, wd (dense, late).
            wq_sb = wp.tile([128, 8, DC], F8E4)
            nc.scalar.dma_start(out=wq_sb, in_=wq8.ap().rearrange("(c p) d -> p c d", p=128))
            wk_sb = wp.tile([128, 8, DC], F8E4)
            nc.sync.dma_start(out=wk_sb, in_=wk8.ap().rearrange("(c p) d -> p c d", p=128))
            if not nobias:
                qkb_sb = consts.tile([1, 2 * DC], mybir.dt.float32r)
                nc.scalar.dma_start(out=qkb_sb, in_=qkb4.ap())
                qb_sb = qkb_sb[:, 0:DC]
                kb_sb = qkb_sb[:, DC:2 * DC]
            if not nobias:
                ones_r = consts.tile([1, 512], mybir.dt.float32r)
                nc.scalar.dma_start(out=ones_r, in_=onesr.ap())
            xb_0 = xsp.tile([128, 8, 512], BF16, tag="xb", name="xb_0")
            nc.sync.dma_start(out=xb_0, in_=xtb.ap().rearrange("(c p) s -> p c s", p=128)[:, :, 0:512])
            wv_sb = wp.tile([128, 8, DC], BF16)
            nc.scalar.dma_start(out=wv_sb, in_=wvb.ap().rearrange("(c p) d -> p c d", p=128))
            band_sb = consts.tile([128, 2048], F8E5)
            nc.scalar.dma_start(out=band_sb, in_=band.ap())
            bb_sb = consts.tile([128, 4, 512], I16)
            nc.scalar.dma_start(out=bb_sb, in_=bb.ap())
            idz_sb = consts.tile([128, 2, 128], F8E4)
            nc.scalar.dma_start(out=idz_sb, in_=idz.ap())
            wd_sb = wp.tile([128, 4, D], BF16)
            nc.scalar.dma_start(out=wd_sb, in_=wdb.ap().rearrange("(c p) o -> p c o", p=128))

            # Q^T/K^T in fp8, partition = 32*(head%4) + d%32,
            # dims (q/k, headgroup, d-half, s)
            qk_sb = qkp.tile([128, 2, 2, 2, S], F8E4)
            q_sb = qk_sb[:, 0]
            k_sb = qk_sb[:, 1]
            # V + ones column per 128-row s-block
            va = vap.tile([128, 16, H, DH + 1], BF16)
            nc.vector.memset(va[:, :, :, DH:DH + 1], 1.0)
            ot = otp.tile([128, 4, S], BF16)   # O^T, row d = 128*dim1 + partition

            def load_x(c):
                a8 = xsp.tile([128, 8, 512], F8E4, tag="x8", name=f"x8_{c}")
                ab = xsp.tile([128, 8, 512], BF16, tag="xb", name=f"xb_{c}")
                nc.sync.dma_start(
                    out=a8, in_=xt8.ap().rearrange("(c p) s -> p c s", p=128)[:, :, 512 * c:512 * (c + 1)])
                nc.sync.dma_start(
                    out=ab, in_=xtb.ap().rearrange("(c p) s -> p c s", p=128)[:, :, 512 * c:512 * (c + 1)])
                return a8, ab

            def proj(c, x8, xb):
                sl = slice(512 * c, 512 * (c + 1))
                for p in range(4):
                    ps = psp.tile([128, 1024], F32, tag="ps", name=f"qk{c}{p}")
                    for i in range(4):
                        nc.tensor.matmul(ps[:, 0:512],
                                         wq_sb[:, 2 * i:2 * i + 2, 128 * p:128 * (p + 1)],
                                         x8[:, 2 * i:2 * i + 2, :],
                                         start=(i == 0), stop=(i == 3), perf_mode=DR)
                    for i in range(4):
                        nc.tensor.matmul(ps[:, 512:1024],
                                         wk_sb[:, 2 * i:2 * i + 2, 128 * p:128 * (p + 1)],
                                         x8[:, 2 * i:2 * i + 2, :],
                                         start=(i == 0), stop=(i == 3), perf_mode=DR)
                    nc.vector.tensor_scalar_add(out=q_sb[:, p // 2, p % 2, sl],
                                                in0=ps[:, 0:512], scalar1=qb_sb[:, p:p + 1])
                    nc.vector.tensor_scalar_add(out=k_sb[:, p // 2, p % 2, sl],
                                                in0=ps[:, 512:1024], scalar1=kb_sb[:, p:p + 1])
                for half in range(2):
                    ps = psp.tile([128, 1024], F32, tag="ps", name=f"v{c}{half}")
                    for g2 in range(2):
                        sb_ = 2 * half + g2
                        for i in range(8):
                            nc.tensor.matmul(ps[:, 512 * g2:512 * (g2 + 1)],
                                             xb[:, i, 128 * sb_:128 * (sb_ + 1)],
                                             wv_sb[:, i, :],
                                             start=(i == 0), stop=(i == 7))
                        nc.gpsimd.tensor_copy(
                            out=va[:, 4 * c + sb_, :, 0:DH],
                            in_=ps[:, 512 * g2:512 * (g2 + 1)].rearrange("p (h d) -> p h d", h=H))

            def mk_scores(c, p, ptc):
                nj = 4 * (c + 1)
                def mk(j):
                    def g():
                        sc = psp.tile([128, 1024], F32, tag="ps", name=f"sc{c}{p}{j}")
                        jj = j - 4 * c          # >= 0 inside the causal band
                        eng = expeng[(c, p, j)]
                        import os
                        clip = 0 if os.environ.get('NO_CLIP') else (128 * jj if jj > 0 else 0)
                        pe_band = jj >= 0 and eng != "dve"
                        for hi in range(2):
                            h = 2 * p + hi
                            gr, hh = h // 4, h % 4
                            lo = 32 * hh
                            nc.tensor.matmul(sc[:, 512 * hi + clip:512 * (hi + 1)],
                                             k_sb[lo:lo + 32, gr, :, 128 * j:128 * (j + 1)],
                                             q_sb[lo:lo + 32, gr, :, 512 * c + clip:512 * (c + 1)],
                                             start=True, stop=not pe_band, perf_mode=DR,
                                             tile_position=(lo, 0))
                            if pe_band:
                                bnd = band_sb[:, 512 * jj + clip:512 * (jj + 1)]
                                nc.tensor.matmul(sc[:, 512 * hi + clip:512 * (hi + 1)],
                                                 idz_sb[:, :, :],
                                                 bnd.unsqueeze(1).broadcast_to(
                                                     [128, 2, 512 - clip]),
                                                 start=False, stop=True, perf_mode=DR)
                        scv = sc.rearrange("p (h q) -> p h q", h=2)[:, :, clip:]
                        ptv = ptc[:, j, :].rearrange("p (h q) -> p h q", h=2)[:, :, clip:]
                        import os
                        if os.environ.get("PT_BF16"):
                            if eng == "act":
                                if clip == 0:
                                    nc.scalar.activation(out=ptc[:, j, :], in_=sc,
                                                         func=AF.Exp, scale=SCALE)
                                else:
                                    nc.scalar.activation(out=ptv, in_=scv,
                                                         func=AF.Exp, scale=SCALE)
                            elif jj >= 0:
                                nc.vector.scalar_tensor_tensor(
                                    out=ptv.bitcast(I16), in0=scv, scalar=SCH_A,
                                    in1=bb_sb[:, jj, clip:].unsqueeze(1).broadcast_to(
                                    [128, 2, 512 - clip]),
                                    op0=ALU.mult, op1=ALU.add)
                            else:
                                nc.vector.tensor_scalar(out=ptv.bitcast(I16), in0=scv,
                                                        scalar1=SCH_A, scalar2=SCH_B,
                                                        op0=ALU.mult, op1=ALU.add)
                            return
                        if eng == "act":
                            nc.scalar.activation(out=ptv.bitcast(BF16), in_=scv,
                                                 func=AF.Exp, scale=SCALE)
                        elif jj >= 0:
                            nc.vector.scalar_tensor_tensor(
                                out=ptv, in0=scv, scalar=SCH_A,
                                in1=bb_sb[:, jj, clip:].unsqueeze(1).broadcast_to(
                                    [128, 2, 512 - clip]),
                                op0=ALU.mult, op1=ALU.add)
                        else:
                            nc.vector.tensor_scalar(out=ptv, in0=scv,
                                                    scalar1=SCH_A, scalar2=SCH_B,
                                                    op0=ALU.mult, op1=ALU.add)
                    return g
                return [mk(j) for j in range(nj)]

            def norm_p(c, p, pvt, osb):
                rs = rsp.tile([128, 2, 2, 2, 1], F32, tag="rs", name=f"rs{c}{p}")
                ov = osb.rearrange("p q (h d) -> p q h d", d=DH)
                for b in range(2):
                    pv_v = pvt[:, b, 0:260].rearrange("p (r h e) -> p r h e", r=2, h=2)
                    rsv = rs[:, b, :, :, :]
                    nc.vector.reciprocal(out=rsv, in_=pv_v[:, :, :, DH:DH + 1])
                    nc.vector.tensor_mul(
                        out=ov[:, 2 * b:2 * b + 2, 2 * p:2 * p + 2, :],
                        in0=pv_v[:, :, :, 0:DH],
                        in1=rsv.broadcast_to([128, 2, 2, DH]))

            def transposes(c, osb):
                for qs in range(4):
                    nc.sync.dma_start_transpose(
                        out=ot[:, :, 512 * c + 128 * qs:512 * c + 128 * (qs + 1)],
                        in_=osb[:, qs, :])

            def transposes_p(c, p, osb):
                for qs in range(4):
                    nc.sync.dma_start_transpose(
                        out=ot[:, p:p + 1, 512 * c + 128 * qs:512 * c + 128 * (qs + 1)],
                        in_=osb[:, qs, 128 * p:128 * (p + 1)])

            def mk_pv(c, p, ptc, osb):
                nj = 4 * (c + 1)
                state = {}
                def mk(j):
                    def g():
                        if j == 0:
                            state["pvt"] = pvp.tile([128, 2, 512], F32, tag="pv",
                                                    name=f"pv{c}{p}")
                        pvt = state["pvt"]
                        jj = j - 4 * c
                        for hi in range(2):
                            h = 2 * p + hi
                            for qs in range(4):
                                import os
                                if (not os.environ.get('NO_CLIP')) and jj > 0 and qs < jj:
                                    continue   # fully-masked block: P == 0
                                b, r = qs // 2, qs % 2
                                off = 130 * r + 65 * hi
                                lhsT_ = ptc[:, j, 512 * hi + 128 * qs:512 * hi + 128 * qs + 128]
                                if not os.environ.get("PT_BF16"):
                                    lhsT_ = lhsT_.bitcast(BF16)
                                nc.tensor.matmul(
                                    pvt[:, b, off:off + 65],
                                    lhsT_,
                                    va[:, j, h, :],
                                    start=(j == 0 and hi == 0 and r == 0),
                                    stop=(j == nj - 1) if os.environ.get("PV_STOP_LAST")
                                         else (j == 4 * c + qs),
                                    skip_group_check=True)
                    return g
                gran = [mk(j) for j in range(nj)]
                def fin():
                    norm_p(c, p, state["pvt"], osb)
                    if p == 3:
                        transposes(c, osb)
                gran.append(fin)
                return gran

            def mk_dense(c):
                def mk(ob2):
                    def g():
                        ps = psp.tile([128, 1024], F32, tag="ps", name=f"d{c}{ob2}")
                        for g2 in range(2):
                            ob = 2 * ob2 + g2
                            for c4 in range(4):
                                nc.tensor.matmul(ps[:, 512 * g2:512 * (g2 + 1)],
                                                 wd_sb[:, c4, 128 * ob:128 * (ob + 1)],
                                                 ot[:, c4, 512 * c:512 * (c + 1)],
                                                 start=(c4 == 0), stop=(c4 == 3))
                            st = dstp.tile([128, 512], F32, tag="dst")
                            if cpeng[("d", c, ob2, g2)] == "act":
                                nc.scalar.activation(out=st,
                                                     in_=ps[:, 512 * g2:512 * (g2 + 1)],
                                                     func=AF.Copy)
                            else:
                                nc.vector.tensor_copy(out=st,
                                                      in_=ps[:, 512 * g2:512 * (g2 + 1)])
                            nc.sync.dma_start(
                                out=yt.ap().rearrange("(c p) s -> p c s", p=128)[:, ob, 512 * c:512 * (c + 1)],
                                in_=st)
                    return g
                return [mk(ob2) for ob2 in range(4)]

            def mk_proj(c, x8, xb):
                def mkqk(p):
                    def g():
                        sl = slice(512 * c, 512 * (c + 1))
                        F32R = mybir.dt.float32r
                        ps = psp.tile([128, 1024], F32, tag="ps", name=f"qk{c}{p}")
                        nob = nobias or bool(os.environ.get("NO_BIAS_MM"))
                        for i in range(4):
                            nc.tensor.matmul(ps[:, 0:512],
                                             wq_sb[:, 2 * i:2 * i + 2, 128 * p:128 * (p + 1)],
                                             x8[:, 2 * i:2 * i + 2, :],
                                             start=(i == 0), stop=(nob and i == 3), perf_mode=DR)
                        if not nob:
                            nc.tensor.matmul(ps[:, 0:512],
                                             qb_sb[:, 128 * p:128 * (p + 1)], ones_r,
                                             start=False, stop=True)
                        for i in range(4):
                            nc.tensor.matmul(ps[:, 512:1024],
                                             wk_sb[:, 2 * i:2 * i + 2, 128 * p:128 * (p + 1)],
                                             x8[:, 2 * i:2 * i + 2, :],
                                             start=(i == 0), stop=(nob and i == 3), perf_mode=DR)
                        if not nob:
                            nc.tensor.matmul(ps[:, 512:1024],
                                             kb_sb[:, 128 * p:128 * (p + 1)], ones_r,
                                             start=False, stop=True)
                        for g2, (dst, srcv) in enumerate(
                                ((q_sb[:, p // 2, p % 2, sl], ps[:, 0:512]),
                                 (k_sb[:, p // 2, p % 2, sl], ps[:, 512:1024]))):
                            if cpeng[("qk", c, p, g2)] == "act":
                                nc.scalar.activation(out=dst, in_=srcv, func=AF.Copy)
                            else:
                                nc.vector.tensor_copy(out=dst, in_=srcv)
                    return g
                def mkv(half):
                    def g():
                        ps = psp.tile([128, 1024], F32, tag="ps", name=f"v{c}{half}")
                        for g2 in range(2):
                            sb_ = 2 * half + g2
                            for i in range(8):
                                nc.tensor.matmul(ps[:, 512 * g2:512 * (g2 + 1)],
                                                 xb[:, i, 128 * sb_:128 * (sb_ + 1)],
                                                 wv_sb[:, i, :],
                                                 start=(i == 0), stop=(i == 7))
                            dst = va[:, 4 * c + sb_, :, 0:DH]
                            srcv = ps[:, 512 * g2:512 * (g2 + 1)].rearrange("p (h d) -> p h d", h=H)
                            if cpeng[("v", c, half, g2)] == "act":
                                nc.scalar.activation(out=dst, in_=srcv, func=AF.Copy)
                            else:
                                nc.vector.tensor_copy(out=dst, in_=srcv)
                    return g
                return [mkqk(p) for p in range(4)] + [mkv(h) for h in range(2)]

            def emit_interleaved(sc_gran, fillers):
                import os
                if os.environ.get("NO_INTERLEAVE"):
                    for g_ in sc_gran: g_()
                    for g_ in fillers: g_()
                    return
                n = len(sc_gran)
                gi = 0
                for j, scg in enumerate(sc_gran):
                    scg()
                    if j >= 1:
                        want = (j * len(fillers)) // max(n - 1, 1)
                        while gi < want:
                            fillers[gi](); gi += 1
                while gi < len(fillers):
                    fillers[gi](); gi += 1

            for gr in mk_proj(0, x8_0, xb_0):
                gr()
            ptcs = {}; osbs = {}
            pending = []
            import os
            import json
            SEQ = json.loads(os.environ.get("SEQ", "[0, 1, 2, 3]"))
            # projections must cover K/V for all chunks <= attended chunk
            projplan = json.loads(os.environ.get("PROJPLAN",
                                  '{"(0, 1)": 1, "(1, 1)": 2, "(2, 1)": 3}'))
            projplan = {tuple(map(int, k.strip("()").split(","))): v
                        for k, v in projplan.items()}
            for s in range(16):
                si, p = divmod(s, 4)
                c = SEQ[si]
                if p == 0:
                    osbs[c] = osbp.tile([128, 4, 512], BF16, tag="osb", name=f"osb{c}")
                _ptdt = BF16 if os.environ.get("PT_BF16") else I16
                ptcs[c] = ptp.tile([128, 16, 1024], _ptdt, tag="pt", name=f"pt{s}")
                fillers = []
                _defer = int(os.environ.get("PV_DEFER", "2"))
                if len(pending) == _defer:
                    fillers += mk_pv(*pending.pop(0))
                if s == 15 and pending:
                    fillers += mk_pv(*pending.pop(0))
                if (si, p) in projplan:
                    cn = projplan[(si, p)]
                    xs_next = load_x(cn)
                    fillers += mk_proj(cn, *xs_next)
                if si > 0:
                    cprev = SEQ[si - 1]
                    if p == 1:
                        fillers += mk_dense(cprev)[0:1]
                    if p == 2:
                        fillers += mk_dense(cprev)[1:2]
                    if p == 3:
                        fillers += mk_dense(cprev)[2:4]
                emit_interleaved(mk_scores(c, p, ptcs[c]), fillers)
                pending.append((c, p, ptcs[c], osbs[c]))
            for pr in pending:
                for gr in mk_pv(*pr):
                    gr()
            for gr in mk_dense(SEQ[3]):
                gr()
    nc.compile()
                return nc
            for s in range(16):
                c, p = divmod(s, 4)
                if p == 0:
                    import os
                    _ptdt = BF16 if os.environ.get("PT_BF16") else I16
                    ptcs[c] = ptp.tile([128, 16, 1024], _ptdt, tag="pt", name=f"pt{c}")
                    osbs[c] = osbp.tile([128, 4, 512], BF16, tag="osb", name=f"osb{c}")
                    if c < 3:
                        xs_next = load_x(c + 1)
                fillers = []
                if len(pending) == 2:
                    fillers += mk_pv(*pending.pop(0))
                if p == 1 and c < 3:
                    fillers += mk_proj(c + 1, *xs_next)
                if p == 3 and c > 0:
                    fillers += mk_dense(c - 1)
                emit_interleaved(mk_scores(c, p, ptcs[c]), fillers)
                pending.append((c, p, ptcs[c], osbs[c]))
            for pr in pending:
                for gr in mk_pv(*pr):
                    gr()
            for gr in mk_dense(3):
                gr()
    nc.compile()
    return nc


def get_nc(nobias=True):
    key = ("nc", nobias)
    if key not in _CACHE:
        _CACHE[key] = _build(nobias)
    return _CACHE[key]


def _perm():
    # new column order: (headgroup g, d-half, head-in-group hh, d%32)
    p = np.empty(DC, np.int64)
    pos = 0
    for g in range(2):
        for half in range(2):
            for hh in range(4):
                for d in range(32):
                    p[pos] = (g * 4 + hh) * DH + half * 32 + d
                    pos += 1
    return p


def kernel(x, mask, wq_w, wq_b, wk_w, wk_b, wv_w, wv_b, dense_w, dense_b,
           _trace=False):
    x = np.asarray(x, dtype=np.float32)
    wq_w = np.asarray(wq_w, np.float32); wq_b = np.asarray(wq_b, np.float32)
    wk_w = np.asarray(wk_w, np.float32); wk_b = np.asarray(wk_b, np.float32)
    wv_w = np.asarray(wv_w, np.float32); wv_b = np.asarray(wv_b, np.float32)
    dense_w = np.asarray(dense_w, np.float32)
    dense_b = np.asarray(dense_b, np.float32)

    perm = _perm()
    # causal band: band[k, 0, 512*jj + q'] = MASKVAL where q' < 128*jj + k
    band = np.zeros((128, 2048), np.float32)
    k_idx = np.arange(128)[:, None]
    q_idx = np.arange(512)[None, :]
    for jj in range(4):
        band[:, 512 * jj:512 * (jj + 1)] = np.where(
            q_idx < 128 * jj + k_idx, MASKVAL, 0.0)
    band = band.astype(E5NP)
    idz = np.zeros((128, 2, 128), np.float32)
    idz[:, 0, :] = np.eye(128, dtype=np.float32)
    idz = idz.astype(E4NP)
    # Schraudolph bias + fused causal mask bits (int16 bf16-bit offsets),
    # duplicated in both 512-col halves (two heads per exp tile)
    bbh = np.empty((128, 4, 512), np.float32)
    q2 = np.arange(512)[None, :]
    for jj in range(4):
        bbh[:, jj, :] = np.where(q2 < 128 * jj + k_idx, SCH_B + MASKVAL * SCH_A, SCH_B)
    bbh = np.round(bbh).astype(np.int16)

    in_maps = []
    for core in range(N_CORES):
        b, hh = divmod(core, 2)
        sl = slice(DC * hh, DC * (hh + 1))
        xt = np.ascontiguousarray(x[b].T)
        in_maps.append({
            "xt8": xt.astype(E4NP),
            "xtb": xt.astype(BFNP),
            "wq8": np.ascontiguousarray((QKSCALE * wq_w[:, sl])[:, perm]).astype(E4NP),
            "wk8": np.ascontiguousarray((QKSCALE * wk_w[:, sl])[:, perm]).astype(E4NP),
            "wvb": np.ascontiguousarray(wv_w[:, sl]).astype(BFNP),
            "wdb": np.ascontiguousarray(dense_w[sl, :]).astype(BFNP),
            "qkb4": np.concatenate([(QKSCALE * wq_b[sl])[perm],
                                    (QKSCALE * wk_b[sl])[perm]])[None, :],
            "band": band, "idz": idz, "bb": bbh,
            "onesr": np.ones((1, 512), np.float32),
        })
    nobias = not (wq_b.any() or wk_b.any())
    nc = get_nc(nobias)
    res = run_bass_kernel_spmd(nc, in_maps, core_ids=list(range(N_CORES)),
                               trace=_trace)
    const = dense_b + wv_b @ dense_w  # bias terms deferred to host
    outs = np.empty((B, S, D), np.float32)
    for b in range(B):
        outs[b] = (res.results[2 * b]["yt"] + res.results[2 * b + 1]["yt"]).T + const
    if _trace:
        kernel.last_result = res
    return outs
